# revision 50
# baseline (speedup 1.0000x reference)
"""Trainium2 Bass kernel for nn_LocalSelfAttention (fused attention block).

Reference (B=2, S=2048, DM=1024, H=16, D=64):
  qkv = x @ Wqkv + bqkv -> split heads -> RoPE(q,k) -> softmax(q k^T/8) v
  -> concat heads @ Wo + bo -> residual + LayerNorm(gamma,beta)

Sharding (8 cores): core c = (batch c//4, query rows 512*(c%4)..+512).
K^T is projected per-core for its OWN 512 positions only and exchanged by a
4-way AllGather per batch replica group (hidden under the V/Q projections);
V is recomputed redundantly -- cheaper than gathering it at the measured
~60 GB/s collective bandwidth.  Attention/out-proj/LN are exact and
row-local; host gather is pure concatenation.

Device layouts: x^T on partitions (host-transposed, bf16); Wq/Wk columns
permuted per head to [evens, odds] so RoPE pairs sit 32 partitions apart;
scores computed transposed (S^T = K Q^T) into [128,1024] PSUM mega-tiles so
each ScalarE Exp covers FD=1024 (amortizes the ~352-cycle ACTIVATE
overhead -- ScalarE exp is the attention-phase floor); the two heads of
each kT/qT tile pair stream interleaved with PV lagging one kc-pair so the
PE never waits on the exp just issued (keeps the HAM clock warm, zero PE
gaps in the attention window); softmax row-sums ride an appended
ones-column on V (M=65), stage into one SBUF row, scatter via a DRAM
bounce onto 16 partitions for ONE batched DVE reciprocal, then broadcast
back via PE outer products against 32-partition-aligned ones rows;
normalization is deferred and applied to bf16 aT tiles.  All matmuls bf16
with fp32 PSUM accumulation; bo is folded into the residual rows on host.
"""
import numpy as np
import ml_dtypes

import concourse.bass as bass
import concourse.mybir as mybir
import concourse.tile as tile
from concourse.bass_utils import run_bass_kernel_spmd

BF16 = ml_dtypes.bfloat16
bf16 = mybir.dt.bfloat16
f32 = mybir.dt.float32
AF = mybir.ActivationFunctionType
ALU = mybir.AluOpType
AX = mybir.AxisListType

B, S, DM = 2, 2048, 1024
H, D = 16, 64
NC = 8
ROWS = S * B // NC          # 512 query rows per core
SB = S
USE_AG = True               # AllGather K-dedup vs fully-redundant K proj


# ---- TileContext tail-drain patch: this walrus rejects >1 sync wait on
# CTRL-class instructions; split the global-clock waits onto SP nops.
def _patched_drain_and_barrier(self, tick_clock, wait_clock):
    nc = self.nc
    drain_inst = nc.sync.drain()
    wait_clock.add_sem_waits(
        drain_inst.ins, tile.ScopedClock({None: tick_clock.global_clock})
    )
    si = drain_inst.ins.sync_info
    waits = list(si.on_wait) if si and si.on_wait else []
    if len(waits) > 1:
        si.on_wait = waits[:1]
        for w in waits[1:]:
            nop = nc.sync.nop()
            nop.ins.sync_info = mybir.SyncInfo(on_wait=[w], on_update=[])
    nc.all_engine_barrier()
    assert self.sems is not None
    popped = nc._tile_sem_poison_stack.pop()
    assert popped is self._sem_poison
    # EVENT_SEMAPHORE_RANGE_CLEAR InstISA trips "ISA wrong length" in this
    # walrus; decrement each sem by its final (compile-time-known) value
    # instead so re-execution of the loaded NEFF starts from zero.
    # (sem decrement via EventSemaphore immediates overflows this walrus's
    # encoding; rely on NRT/PJRT resetting sem state between executions --
    # verified by the second-call check in test.py)
    nc.all_engine_barrier()


tile.TileContext._drain_and_barrier = _patched_drain_and_barrier

_CTRL_CLASSES = ("InstNoOp", "InstDrain", "InstEventSemaphore")


def _split_excess_waits(nc, maxw_compute=1):
    """Walrus (this version) caps sync waits per instruction (1 for
    CTRL-class, ~2 for compute).  Hoist excess waits onto same-engine NoOps
    inserted immediately before the offending instruction."""
    import copy
    proto = nc.sync.nop().ins  # prototype NoOp (appended to current bb; harmless)
    proto_si = proto.sync_info
    if proto_si and proto_si.on_wait:
        proto.sync_info = mybir.SyncInfo(on_wait=[], on_update=[])
    nsplit = 0
    for f in nc.m.functions:
        for b in f.blocks:
            insts = list(b.instructions)
            out = []
            changed = False
            for inst in insts:
                cls = type(inst).__name__
                maxw = 1 if cls in _CTRL_CLASSES else maxw_compute
                si = inst.sync_info
                waits = list(si.on_wait) if si and si.on_wait else []
                if len(waits) > maxw:
                    keep = waits[:maxw]
                    extra = waits[maxw:]
                    si.on_wait = keep
                    for i, w in enumerate(extra):
                        nop = copy.deepcopy(proto)
                        nop.name = f"{inst.name}-wsplit{i}"
                        nop.engine = inst.engine
                        nop.sync_info = mybir.SyncInfo(on_wait=[w],
                                                       on_update=[])
                        out.append(nop)
                        nsplit += 1
                    changed = True
                out.append(inst)
            if changed:
                try:
                    b.instructions = out
                except Exception:
                    b.set_instructions(out)
    return nsplit


def _build_program():
    nc = bass.Bass("TRN2", target_bir_lowering=False, debug=False,
                   num_devices=NC)

    def din(name, shape, dt):
        return nc.dram_tensor(name, list(shape), dt, kind="ExternalInput").ap()

    xT = din("xT", (DM, SB), bf16)
    xTq = din("xTq", (DM, ROWS), bf16)
    xr = din("xr", (ROWS, DM), f32)          # x rows + bo (folded on host)
    wq = din("wq", (DM, DM), bf16)
    wk = din("wk", (DM, DM), bf16)
    wv = din("wv", (DM, DM), bf16)
    wo = din("wo", (DM, DM), bf16)
    ccr = din("ccr", (128, ROWS), bf16)
    ssr = din("ssr", (128, ROWS), bf16)
    bqp = din("bqp", (128, 8), f32)
    bkp = din("bkp", (128, 8), f32)
    bvn = din("bvn", (128, 8), f32)
    gbc = din("gbc", (128, DM), bf16)
    bbc = din("bbc", (128, DM), bf16)
    out = nc.dram_tensor("out", [ROWS, DM], f32, kind="ExternalOutput").ap()
    rs_dram = [nc.dram_tensor(f"rs_stage{g}", [1, 8 * 512], bf16,
                              kind="Internal").ap() for g in range(2)]
    rinv_dram = [nc.dram_tensor(f"rinv_stage{g}", [8, 512], bf16,
                                kind="Internal").ap() for g in range(2)]

    with tile.TileContext(nc) as tc:
        with tc.tile_pool(name="res", bufs=1) as res, \
             tc.tile_pool(name="tmp", bufs=3) as tmp, \
             tc.tile_pool(name="ppool", bufs=5) as ppool:

            xt_sb = [res.tile([128, SB], bf16, name=f"xt{k}", tag=f"xt{k}") for k in range(8)]
            xq_sb = [res.tile([128, ROWS], bf16, name=f"xq{k}", tag=f"xq{k}") for k in range(8)]
            kT = [res.tile([128, SB], bf16, name=f"kT{t}", tag=f"kT{t}") for t in range(8)]
            qT = [res.tile([128, ROWS], bf16, name=f"qT{t}", tag=f"qT{t}") for t in range(8)]
            vt = [res.tile([128, H * (D + 1)], bf16, name=f"vt{m}", tag=f"vt{m}")
                  for m in range(16)]
            aT = [res.tile([128, ROWS], bf16, name=f"aT{t}", tag=f"aT{t}") for t in range(8)]
            ccr_sb = res.tile([128, ROWS], bf16, tag="ccr")
            ssr_sb = res.tile([128, ROWS], bf16, tag="ssr")
            bq_sb = res.tile([128, 8], f32, tag="bq")
            bk_sb = res.tile([128, 8], f32, tag="bk")
            bv_sb = res.tile([128, 8], f32, tag="bv")
            eps_sb = res.tile([128, 1], f32, tag="eps")

            # load order: everything K-proj needs first (xq, tables, biases)
            # so the AllGather triggers as early as possible; the V-proj
            # inputs (xt) stream in behind it.
            for k in range(8):
                nc.sync.dma_start(xq_sb[k][:], xTq[k * 128:(k + 1) * 128, :])
            nc.sync.dma_start(ccr_sb[:], ccr[:])
            nc.sync.dma_start(ssr_sb[:], ssr[:])
            nc.sync.dma_start(bq_sb[:], bqp[:])
            nc.sync.dma_start(bk_sb[:], bkp[:])
            nc.sync.dma_start(bv_sb[:], bvn[:])
            nc.vector.memset(eps_sb[:], 1e-5)

            def rope(dst, src, cct, sst, n0, nn):
                # dst[:, n0:n0+nn] = src*CC + swap32(src)*SS
                # (cross-partition 2-input DVE ops are illegal -> copy first)
                t1 = tmp.tile([128, nn], bf16, tag="ropet1")
                t2 = tmp.tile([128, nn], bf16, tag="ropet2")
                for a, b_ in ((0, 32), (32, 0), (64, 96), (96, 64)):
                    nc.vector.tensor_copy(t2[a:a + 32, :], src[b_:b_ + 32, :])
                nc.vector.tensor_tensor(out=t1[:], in0=src[:],
                                        in1=cct[:, n0:n0 + nn], op=ALU.mult)
                nc.vector.tensor_tensor(out=t2[:], in0=t2[:],
                                        in1=sst[:, n0:n0 + nn], op=ALU.mult)
                nc.vector.tensor_tensor(out=dst[:, n0:n0 + nn], in0=t1[:],
                                        in1=t2[:], op=ALU.add)

            # ---- projections ----
            # Each core projects K only for its OWN 512 positions; a 4-way
            # AllGather (per batch replica group) exchanges the RoPEd K^T
            # blocks while the PE does the (redundant) V projection and Q.
            # V stays redundant: recomputing it (41us PE) beats gathering it
            # (~64us exposed collective).  SPMD-uniform: the own K block
            # round-trips through the gather output too.  The gathered-K
            # unpack DMAs issue from the gpsimd queue AFTER the collective
            # instruction -- the collective blocks that queue until complete
            # (DRAM tiles are not dependency-tracked, so sync-queue DMAs
            # would race it).
            RG = [[0, 1, 2, 3], [4, 5, 6, 7]]
            with tc.tile_pool(name="dram", bufs=1, space="DRAM") as dpool, \
                 tc.tile_pool(name="wts", bufs=1) as wts, \
                 tc.tile_pool(name="psP", bufs=2, space="PSUM") as psP:
                kin_a = dpool.tile([512, 512], bf16, name="kin_a")
                kin_b = dpool.tile([512, 512], bf16, name="kin_b")
                kout_a = dpool.tile([2048, 512], bf16, name="kout_a")
                kout_b = dpool.tile([2048, 512], bf16, name="kout_b")

                wk_sb = [wts.tile([128, DM], bf16, name=f"wk{k}", tag=f"wk{k}")
                         for k in range(8)]
                wv_sb = [wts.tile([128, DM], bf16, name=f"wv{k}", tag=f"wv{k}")
                         for k in range(8)]
                kT_own = [wts.tile([128, 512], bf16, name=f"ko{t}",
                                   tag=f"ko{t}") for t in range(8)]
                for k in range(8):
                    nc.sync.dma_start(wk_sb[k][:], wk[k * 128:(k + 1) * 128, :])

                # K^T projection (own 512 positions) + RoPE, then AllGather
                for t in range(8):
                    ps = psP.tile([128, 512], f32, tag="proj")
                    for kd in range(8):
                        nc.tensor.matmul(
                            ps[:], wk_sb[kd][:, t * 128:(t + 1) * 128],
                            xq_sb[kd][:], start=(kd == 0), stop=(kd == 7))
                    kt_raw = tmp.tile([128, 512], bf16, tag="evac")
                    nc.scalar.activation(kt_raw[:], ps[:], AF.Identity,
                                         bias=bk_sb[:, t:t + 1])
                    rope(kT_own[t], kt_raw, ccr_sb, ssr_sb, 0, 512)
                    kin_t, tt = (kin_a, t) if t < 4 else (kin_b, t - 4)
                    nc.sync.dma_start(kin_t[tt * 128:(tt + 1) * 128, :],
                                      kT_own[t][:])
                    # split gather: the t=0-3 half triggers 4 tiles earlier,
                    # unblocking the first half of attention sooner; the
                    # second half hides under attention t=0-3.
                    if t == 3:
                        cc_ka = nc.gpsimd.collective_compute(
                            "AllGather", ALU.bypass, replica_groups=RG,
                            ins=[kin_a.opt()], outs=[kout_a.opt()])
                    if t == 7:
                        cc_kb = nc.gpsimd.collective_compute(
                            "AllGather", ALU.bypass, replica_groups=RG,
                            ins=[kin_b.opt()], outs=[kout_b.opt()])

                # Q^T projection + RoPE (wq reuses wk slots)
                wq_sb = [wts.tile([128, DM], bf16, name=f"wq{k}", tag=f"wk{k}")
                         for k in range(8)]
                for k in range(8):
                    nc.sync.dma_start(wq_sb[k][:], wq[k * 128:(k + 1) * 128, :])
                # only t=0,1 upfront; qT[t] for t>=2 is projected in small
                # bursts between attention iterations (it is not needed
                # until attention-t starts), pulling ~100 PE-matmuls off the
                # pre-attention critical path.
                for t in range(2):
                    ps = psP.tile([128, 512], f32, tag="proj")
                    for kd in range(8):
                        nc.tensor.matmul(
                            ps[:], wq_sb[kd][:, t * 128:(t + 1) * 128],
                            xq_sb[kd][:], start=(kd == 0), stop=(kd == 7))
                    q_raw = tmp.tile([128, ROWS], bf16, tag="evac")
                    nc.scalar.activation(q_raw[:], ps[:], AF.Identity,
                                         bias=bq_sb[:, t:t + 1])
                    rope(qT[t], q_raw, ccr_sb, ssr_sb, 0, ROWS)

                # V projection (redundant, all 2048 positions; 65-stride
                # head slots + ones column)
                for k in range(8):
                    nc.sync.dma_start(xt_sb[k][:], xT[k * 128:(k + 1) * 128, :])
                    nc.sync.dma_start(wv_sb[k][:], wv[k * 128:(k + 1) * 128, :])
                for m in range(16):
                    m0 = m * 128
                    for ncol in range(2):
                        c0 = ncol * 512
                        ps = psP.tile([128, 512], f32, tag="proj")
                        for kd in range(8):
                            nc.tensor.matmul(
                                ps[:], xt_sb[kd][:, m0:m0 + 128],
                                wv_sb[kd][:, c0:c0 + 512],
                                start=(kd == 0), stop=(kd == 7))
                        dst = vt[m][:, ncol * 8 * 65:(ncol + 1) * 8 * 65]
                        dstv = dst.rearrange("p (h e) -> p h e", e=65)[:, :, 0:64]
                        srcv = ps[:].rearrange("p (h e) -> p h e", e=64)
                        nc.scalar.activation(dstv, srcv, AF.Identity)
                    onev = vt[m][:, :].rearrange("p (h e) -> p h e",
                                                 e=65)[:, :, 64:65]
                    nc.vector.memset(onev, 1.0)

                # gathered K^T -> attention layout.  Emitted after the wq
                # loads so the sync-queue head-of-line wait on the gather
                # doesn't starve the Q projection.  DRAM tiles are not
                # dependency-tracked, so attach explicit edges to the
                # collectives.
                for g, (kout_g, cc_g) in enumerate(
                        ((kout_a, cc_ka), (kout_b, cc_kb))):
                    for i in range(4):
                        for tt in range(4):
                            t = g * 4 + tt
                            dma = nc.sync.dma_start(
                                kT[t][:, i * 512:(i + 1) * 512],
                                kout_g[i * 512 + tt * 128:
                                       i * 512 + (tt + 1) * 128, :])
                            bass._add_dep_helper(dma.ins, cc_g.ins, sync=True,
                                                 reason="AG_K output read")

            # ---- attention ----
            # scores transposed (S^T = K^T-chunk @ Q^T) into [128,1024] PSUM
            # mega-tiles (2 banks) so each Exp covers FD=1024: the Act
            # engine's 352-cycle per-instruction overhead is the phase
            # bottleneck.  Normalization deferred: aT holds unnormalized
            # attn (bf16); rowsum rows stage into one SBUF row, one DMA
            # scatters them across 16 partitions, ONE batched reciprocal,
            # a second scatter puts reciprocals at partitions {0,32} for the
            # PE broadcast outer-products (lhsT/rhs bases must be 32-aligned).
                rs_stage = [wts.tile([65, 512], bf16, tag=f"rs_st{p}",
                                     name=f"rs_st{p}") for p in range(2)]
                rs8 = [wts.tile([8, 512], bf16, tag=f"rs8_{g}",
                                name=f"rs8_{g}") for g in range(2)]
                rinv8 = [wts.tile([8, 512], bf16, tag=f"rinv8_{g}",
                                  name=f"rinv8_{g}") for g in range(2)]
                rinvA = wts.tile([33, 8 * 512], bf16, tag="rinvA")
                onesA = wts.tile([33, 64], bf16, tag="onesA")
                nc.vector.memset(onesA[:], 1.0)

                # Two heads of a t-pair stream interleaved: doubles the
                # independent PE work between each Exp and its PV consumers,
                # keeping the PE dense (HAM-warm) while Act saturates.
                with tc.tile_pool(name="psA", bufs=2, space="PSUM") as psA, \
                     tc.tile_pool(name="psO", bufs=2, space="PSUM") as psO:
                    for t in range(8):
                        oaccs = [psO.tile([65, 512], f32, tag="oacc",
                                          name=f"oacc{t}_{hh}")
                                 for hh in range(2)]
                        prev = [None, None]

                        def emit_pv(hh, kp, pT_t):
                            h = 2 * t + hh
                            for j in range(2):
                                kc = kp * 2 + j
                                nc.tensor.matmul(
                                    oaccs[hh][:],
                                    vt[kc][:, h * 65:h * 65 + 65],
                                    pT_t[:, j * 512:(j + 1) * 512],
                                    start=(kc == 0), stop=(kc == 15))

                        for kp in range(8):
                            for hh in range(2):
                                po = 64 * hh
                                sps = psA.tile([128, 1024], f32, tag="sco")
                                for j in range(2):
                                    kc = kp * 2 + j
                                    nc.tensor.matmul(
                                        sps[:, j * 512:(j + 1) * 512],
                                        kT[t][po:po + 64,
                                              kc * 128:(kc + 1) * 128],
                                        qT[t][po:po + 64, :],
                                        start=True, stop=True)
                                pT = ppool.tile([128, 1024], bf16, tag="pT")
                                nc.scalar.activation(pT[:], sps[:], AF.Exp,
                                                     scale=0.125)
                                if prev[hh] is not None:
                                    emit_pv(hh, kp - 1, prev[hh])
                                prev[hh] = pT
                        for hh in range(2):
                            emit_pv(hh, 7, prev[hh])
                        # stash rowsum rows (same-partition copies) + unnorm.
                        # attn (out-partition shift legal for 1-input copies)
                        for hh in range(2):
                            h, po = 2 * t + hh, 64 * hh
                            nc.vector.tensor_copy(rs_stage[hh][64:65, :],
                                                  oaccs[hh][64:65, :])
                            nc.sync.dma_start(
                                rs_dram[h // 8][:, (h % 8) * 512:
                                                (h % 8 + 1) * 512],
                                rs_stage[hh][64:65, :])
                            nc.vector.tensor_copy(aT[t][po:po + 64, :],
                                                  oaccs[hh][0:64, :])
                        # reciprocal chains in two batches so their DRAM
                        # bounce + recip latency hides under the remaining
                        # attention work: gather rowsums onto 8 partitions,
                        # one batched reciprocal, scatter to partitions
                        # {0,32} (head 2g+i -> partition 32i, col g*512).
                        if t in (3, 7):
                            g8 = 0 if t == 3 else 1
                            c0 = g8 * 4096
                            nc.sync.dma_start(
                                rs8[g8][:],
                                rs_dram[g8].rearrange("a (p c) -> (a p) c",
                                                      p=8))
                            with nc.allow_low_precision(
                                    reason="softmax 1/rowsum in bf16"):
                                nc.vector.reciprocal(rinv8[g8][:],
                                                     rs8[g8][:])
                            nc.sync.dma_start(rinv_dram[g8][:], rinv8[g8][:])
                            for i in range(2):
                                nc.sync.dma_start(
                                    rinvA[32 * i:32 * i + 1,
                                          c0 // 2:c0 // 2 + 2048].rearrange(
                                        "a (g c) -> a g c", c=512),
                                    rinv_dram[g8].rearrange(
                                        "(g i) c -> i g c", i=2)[i:i + 1])
                        # project qT[t+2] in a short burst (8 MMs) between
                        # attention iterations; bias-add evac on the DVE so
                        # the Act exp stream is untouched
                        if t + 2 < 8:
                            tq = t + 2
                            psq = psP.tile([128, 512], f32, tag="proj")
                            for kd in range(8):
                                nc.tensor.matmul(
                                    psq[:],
                                    wq_sb[kd][:, tq * 128:(tq + 1) * 128],
                                    xq_sb[kd][:],
                                    start=(kd == 0), stop=(kd == 7))
                            q_raw = tmp.tile([128, ROWS], bf16, tag="evac")
                            nc.vector.tensor_scalar(
                                out=q_raw[:], in0=psq[:],
                                scalar1=bq_sb[:, tq:tq + 1], scalar2=None,
                                op0=ALU.add)
                            rope(qT[tq], q_raw, ccr_sb, ssr_sb, 0, ROWS)

                with tc.tile_pool(name="psB", bufs=2, space="PSUM") as psB:
                    for h in range(H):
                        t, po = h // 2, 64 * (h % 2)
                        g, i = h // 2, h % 2
                        bc = psB.tile([128, 512], f32, tag="bc")
                        nc.tensor.matmul(bc[po:po + 64, :],
                                         onesA[32 * i:32 * i + 1, :],
                                         rinvA[32 * i:32 * i + 1,
                                               g * 512:(g + 1) * 512],
                                         start=True, stop=True)
                        nc.vector.tensor_tensor(out=aT[t][po:po + 64, :],
                                                in0=aT[t][po:po + 64, :],
                                                in1=bc[po:po + 64, :],
                                                op=ALU.mult)
                        nc.vector.tensor_scalar(
                            out=aT[t][po:po + 64, :],
                            in0=aT[t][po:po + 64, :],
                            scalar1=bv_sb[po:po + 64, h // 2:h // 2 + 1],
                            scalar2=None, op0=ALU.add)

            # ---- out-proj + residual + LayerNorm ----
            with tc.tile_pool(name="wop", bufs=1) as wop, \
                 tc.tile_pool(name="fin", bufs=2) as fin, \
                 tc.tile_pool(name="psF", bufs=2, space="PSUM") as psF:
                wo_sb = [wop.tile([128, DM], bf16, name=f"wo{k}", tag=f"wo{k}")
                         for k in range(8)]
                for k in range(8):
                    nc.sync.dma_start(wo_sb[k][:], wo[k * 128:(k + 1) * 128, :])
                g_sb = wop.tile([128, DM], bf16, tag="g")
                b_sb = wop.tile([128, DM], bf16, tag="b")
                nc.sync.dma_start(g_sb[:], gbc[:])
                nc.sync.dma_start(b_sb[:], bbc[:])

                for mr in range(4):
                    rr = mr * 128
                    xb = fin.tile([128, DM], f32, tag="xb")
                    nc.sync.dma_start(xb[:], xr[rr:rr + 128, :])
                    # LN arithmetic in bf16: 2x/4x DVE modes; abs error
                    # ~2e-3 of a ~5.0-max output, well inside tolerance
                    hrow = fin.tile([128, DM], bf16, tag="hrow")
                    for ncol in range(2):
                        c0 = ncol * 512
                        ps = psF.tile([128, 512], f32, tag="fin")
                        for kd in range(8):
                            nc.tensor.matmul(
                                ps[:], aT[kd][:, rr:rr + 128],
                                wo_sb[kd][:, c0:c0 + 512],
                                start=(kd == 0), stop=(kd == 7))
                        nc.vector.tensor_tensor(
                            out=hrow[:, c0:c0 + 512], in0=ps[:],
                            in1=xb[:, c0:c0 + 512], op=ALU.add)
                    ssum = fin.tile([128, 1], f32, tag="ssum")
                    nc.vector.reduce_sum(out=ssum[:], in_=hrow[:], axis=AX.X)
                    mu = fin.tile([128, 1], f32, tag="mu")
                    nc.vector.tensor_scalar(out=mu[:], in0=ssum[:],
                                            scalar1=1.0 / DM, scalar2=None,
                                            op0=ALU.mult)
                    d = fin.tile([128, DM], bf16, tag="d")
                    nc.vector.tensor_scalar(out=d[:], in0=hrow[:],
                                            scalar1=mu[:], scalar2=None,
                                            op0=ALU.subtract)
                    y = fin.tile([128, DM], bf16, tag="y")
                    vs = fin.tile([128, 1], f32, tag="vs")
                    nc.vector.tensor_tensor(out=y[:], in0=d[:], in1=d[:],
                                            op=ALU.mult)
                    nc.vector.reduce_sum(out=vs[:], in_=y[:], axis=AX.X)
                    st = fin.tile([128, 1], f32, tag="st")
                    nc.scalar.activation(st[:], vs[:], AF.Sqrt,
                                         bias=eps_sb[:], scale=1.0 / DM)
                    rstd = fin.tile([128, 1], f32, tag="rstd")
                    nc.vector.reciprocal(rstd[:], st[:])
                    nc.vector.tensor_scalar(out=y[:], in0=d[:],
                                            scalar1=rstd[:], scalar2=None,
                                            op0=ALU.mult)
                    nc.vector.tensor_tensor(out=y[:], in0=y[:], in1=g_sb[:],
                                            op=ALU.mult)
                    yf = fin.tile([128, DM], f32, tag="yf")
                    nc.vector.tensor_tensor(out=yf[:], in0=y[:], in1=b_sb[:],
                                            op=ALU.add)
                    nc.sync.dma_start(out[rr:rr + 128, :], yf[:])

    _split_excess_waits(nc)
    return nc


_NC_CACHE = None


def _perm():
    p = np.zeros(DM, np.int64)
    for h in range(H):
        p[h * D:h * D + 32] = h * D + np.arange(0, D, 2)
        p[h * D + 32:(h + 1) * D] = h * D + np.arange(1, D, 2)
    return p


def kernel(x, Wqkv, bqkv, Wo, bo, gamma, beta):
    global _NC_CACHE
    x = np.asarray(x, np.float32)
    Wqkv = np.asarray(Wqkv, np.float32)
    bqkv = np.asarray(bqkv, np.float32)
    Wo = np.asarray(Wo, np.float32)
    bo = np.asarray(bo, np.float32)
    gamma = np.asarray(gamma, np.float32)
    beta = np.asarray(beta, np.float32)

    perm = _perm()
    Wq = np.ascontiguousarray(Wqkv[:, 0:DM][:, perm]).astype(BF16)
    Wk = np.ascontiguousarray(Wqkv[:, DM:2 * DM][:, perm]).astype(BF16)
    Wv = np.ascontiguousarray(Wqkv[:, 2 * DM:3 * DM]).astype(BF16)
    Wob = Wo.astype(BF16)
    bq = bqkv[0:DM][perm]
    bk = bqkv[DM:2 * DM][perm]
    bv = bqkv[2 * DM:3 * DM]

    inv = 1.0 / (10000.0 ** (np.arange(0, D, 2, dtype=np.float64) / D))
    pos = np.arange(S, dtype=np.float64)
    fr = pos[None, :] * inv[:, None]                    # [32, S]
    c32, s32 = np.cos(fr), np.sin(fr)
    CC = np.concatenate([c32, c32, c32, c32], 0).astype(BF16)   # [128, S]
    SS = np.concatenate([-s32, s32, -s32, s32], 0).astype(BF16)

    def colmajor(v):
        return np.ascontiguousarray(v.reshape(8, 128).T).astype(np.float32)

    gB = np.ascontiguousarray(np.broadcast_to(gamma, (128, DM))).astype(BF16)
    bB = np.ascontiguousarray(np.broadcast_to(beta, (128, DM))).astype(BF16)

    if _NC_CACHE is None:
        _NC_CACHE = _build_program()
    nc = _NC_CACHE

    in_maps = []
    for c in range(NC):
        b, r = c // 4, c % 4
        xTb = np.ascontiguousarray(x[b].T).astype(BF16)
        rr = r * ROWS
        in_maps.append({
            "xT": xTb,
            "xTq": np.ascontiguousarray(xTb[:, rr:rr + ROWS]),
            "xr": np.ascontiguousarray(x[b, rr:rr + ROWS, :] + bo[None, :]),
            "wq": Wq, "wk": Wk, "wv": Wv, "wo": Wob,
            "ccr": np.ascontiguousarray(CC[:, rr:rr + ROWS]),
            "ssr": np.ascontiguousarray(SS[:, rr:rr + ROWS]),
            "bqp": colmajor(bq), "bkp": colmajor(bk), "bvn": colmajor(bv),
            "gbc": gB, "bbc": bB,
        })

    res = run_bass_kernel_spmd(nc, in_maps, core_ids=list(range(NC)))
    kernel._last_results = res
    full = np.empty((B, S, DM), np.float32)
    for c in range(NC):
        b, r = c // 4, c % 4
        full[b, r * ROWS:(r + 1) * ROWS, :] = res.results[c]["out"]
    return full

